# revision 8
# baseline (speedup 1.0000x reference)
"""4-layer GCN (EnhancedGCN) on 8 Trainium2 NeuronCores.

Strategy (node/graph parallel):
  - Nodes sharded 12500/core across 8 cores; edges assigned to the core
    owning their dst node.
  - Per layer, h (pre-scaled by norm_src, fp16) is replicated via FOUR
    chunked AllGathers (chunk b = rows [b*3125,(b+1)*3125) of every core's
    shard -> one 25000-row "bank" tensor); chunk AGs are issued as soon as
    the producing windows finish, so they overlap tail-window compute.
  - Each core gathers the src rows for its edges with dma_gather, one call
    per (superstep of 4 dst windows) x bank (2560 idx / call) to amortize
    the ~1us SWDGE fixed cost; indices are sorted ascending within each
    (window,bank) group for DRAM page locality.
  - Aggregation per 128-node dst window: one-hot matmuls (fp16 msg x fp8
    one-hot slab streamed from HBM) accumulated in PSUM, then the dense W
    matmul; norm_dst*z + b on DVE; GELU + PSUM copy on the scalar (ACT)
    engine to keep DVE slack; final LayerNorm on DVE.
  - Graph preprocessing (degree norms, edge grouping with a uniform
    subtile count, padding, gather index layout) happens on host once; the
    compiled program is shared by all 8 cores (SPMD).
"""

import sys
import types

import numpy as np

N_NODES = 100000
N_EDGES = 1600000
D = 128
NCORES = 8
NPC = N_NODES // NCORES            # 12500 nodes per core
WINDOWS = (NPC + 127) // 128       # 98 dst windows per core (last has 84 rows)
BANKS = 4
CHUNK = NPC // BANKS               # 3125 rows per AG chunk per core
BANK_ROWS = CHUNK * NCORES         # 25000 rows per bank tensor (int16-safe)
import os as _os
K_SS = int(_os.environ.get("KSS", "4"))  # dst windows per superstep (gather batch)
AG_LATE = _os.environ.get("AGLATE", "") != ""  # issue chunk AGs only at layer end
PAD_DLOC = 999.0

TRACE = False
LAST_EXEC_NS = None

_CACHE = {}


def _install_ntff_hook():
    if "antenv.axon_hooks" in sys.modules:
        return
    mod = types.ModuleType("antenv.axon_hooks")
    _hook = [None]
    mod.set_axon_ntff_profile_hook = lambda h: _hook.__setitem__(0, h)
    mod.get_axon_ntff_profile_hook = lambda: _hook[0]
    sys.modules["antenv.axon_hooks"] = mod
    import antenv

    antenv.axon_hooks = mod
    try:
        from trn_agent_boot.trn_boot import _ntff_profile_via_ctypes

        mod.set_axon_ntff_profile_hook(
            _ntff_profile_via_ctypes("/opt/axon/libaxon_pjrt.so")
        )
    except Exception:
        pass


def _superstep_sizes():
    sizes = []
    w = 0
    while w < WINDOWS:
        k = min(K_SS, WINDOWS - w)
        sizes.append(k)
        w += k
    return sizes


def _prep_graph(src, dst):
    """Host-side graph preprocessing shared by all layers."""
    src = np.asarray(src).astype(np.int64).ravel()
    dst = np.asarray(dst).astype(np.int64).ravel()

    deg_src = np.bincount(src, minlength=N_NODES).astype(np.float64)
    deg_dst = np.bincount(dst, minlength=N_NODES).astype(np.float64)
    norm_src = np.clip(deg_src, 1.0, None) ** -0.5
    norm_dst = np.clip(deg_dst, 1.0, None) ** -0.5

    # src node -> (bank, row-in-bank) under the chunked-AG layout:
    # bank b holds rows [c*3125+(r%3125)] = AllGather of every core's slice b
    s_core = src // NPC
    s_rem = src % NPC
    s_bank = s_rem // CHUNK
    s_row = s_core * CHUNK + (s_rem % CHUNK)   # 0..24999

    core = dst // NPC
    w = (dst % NPC) // 128
    group = (core * WINDOWS + w) * BANKS + s_bank
    order = np.argsort(group, kind="stable")
    g_sorted = group[order]
    row_sorted = s_row[order]
    dst_sorted = dst[order]

    n_groups = NCORES * WINDOWS * BANKS
    counts = np.bincount(g_sorted, minlength=n_groups)
    starts = np.zeros(n_groups + 1, np.int64)
    np.cumsum(counts, out=starts[1:])

    S = int(np.ceil(counts.max() / 128.0))   # uniform subtiles per (w,b)
    CAP = S * 128

    sizes = _superstep_sizes()
    # idx col offsets: [ss][b] -> start col in idx16 (16 idx per col)
    off_idx = []
    icol = 0
    for k in sizes:
        row_b = []
        for b in range(BANKS):
            row_b.append(icol)
            icol += k * CAP // 16
        off_idx.append(row_b)
    total_idxcols = icol
    # s8 col offsets: [ss] -> start sub; subs within ss: (wl*BANKS+b)*S+s
    off_sub = []
    scol = 0
    for k in sizes:
        off_sub.append(scol)
        scol += k * BANKS * S
    total_subs = scol

    per_core = []
    for c in range(NCORES):
        idx16 = np.zeros((128, total_idxcols), np.int16)
        dloc = np.full((128, total_subs), PAD_DLOC, np.float16)
        for si, k in enumerate(sizes):
            for b in range(BANKS):
                icol0 = off_idx[si][b]
                for wl in range(k):
                    wi = si * K_SS + wl
                    gidx = (c * WINDOWS + wi) * BANKS + b
                    s0, s1 = starts[gidx], starts[gidx + 1]
                    n_e = s1 - s0
                    loc = np.zeros(CAP, np.int64)
                    dl = np.full(CAP, PAD_DLOC, np.float64)
                    # ascending src rows within the group: DRAM locality
                    loc[:n_e] = row_sorted[s0:s1]
                    dl[:n_e] = (dst_sorted[s0:s1] % NPC) - wi * 128
                    # idx layout: index i -> partition i%16, col i//16,
                    # replicated across the 8 partition stripes
                    stripe = loc.reshape(CAP // 16, 16).T.astype(np.int16)
                    cc0 = icol0 + wl * (CAP // 16)
                    for st in range(8):
                        idx16[16 * st:16 * st + 16, cc0:cc0 + CAP // 16] = stripe
                    # subtile layout: edge i -> partition i%128, subtile i//128
                    sub0 = off_sub[si] + (wl * BANKS + b) * S
                    dloc[:, sub0:sub0 + S] = dl.reshape(S, 128).T.astype(np.float16)
        onehot = (
            dloc[:, :, None] == np.arange(128, dtype=np.float16)[None, None, :]
        )
        import ml_dtypes
        s8 = onehot.astype(ml_dtypes.float8_e4m3).reshape(128, total_subs * 128)
        per_core.append((idx16, s8))

    def node_tile(vec, c):
        full = np.zeros(WINDOWS * 128, np.float32)
        full[:NPC] = vec[c * NPC:(c + 1) * NPC].astype(np.float32)
        return full.reshape(WINDOWS, 128).T.copy()

    ns_tiles = [node_tile(norm_src, c) for c in range(NCORES)]
    ndn_tiles = [node_tile(norm_dst, c) for c in range(NCORES)]

    return S, off_idx, off_sub, total_idxcols, total_subs, per_core, \
        ns_tiles, ndn_tiles


# AG chunk b is complete once windows 0..AG_BOUNDARY[b] have been written.
AG_BOUNDARY = [
    ((b + 1) * CHUNK + 127) // 128 - 1 for b in range(BANKS)
]  # [24, 48, 73, 97]


def _build_program(S, off_idx, off_sub, total_idxcols, total_subs):
    import os

    import concourse.bacc as bacc
    import concourse.mybir as mybir
    import concourse.tile as tile

    dbg_layers = int(os.environ.get("DBG_LAYERS", "4"))
    CAP = S * 128
    sizes = _superstep_sizes()

    nc = bacc.Bacc(
        "TRN2",
        target_bir_lowering=False,
        debug=False,
        enable_asserts=False,
        num_devices=NCORES,
        num_swdge_queues=BANKS,
        # headroom over the default 16 KiB carveout (1024 descriptors) so
        # back-to-back 1024-idx gathers on one queue never wait for reclaim
        dynamic_dma_scratch_size=32768,
    )
    f32, f16, i16 = mybir.dt.float32, mybir.dt.float16, mybir.dt.int16
    f8 = mybir.dt.float8e4

    x_in = nc.dram_tensor("x", [NPC, D], f32, kind="ExternalInput")
    idx_in = nc.dram_tensor("idx16", [128, total_idxcols], i16, kind="ExternalInput")
    s8_in = nc.dram_tensor("s8", [128, total_subs * D], f8, kind="ExternalInput")
    ns_in = nc.dram_tensor("ns", [128, WINDOWS], f32, kind="ExternalInput")
    ndn_in = nc.dram_tensor("ndn", [128, WINDOWS], f32, kind="ExternalInput")
    w_in = [nc.dram_tensor(f"W{i+1}", [D, D], f16, kind="ExternalInput") for i in range(4)]
    bb_in = [nc.dram_tensor(f"bb{i+1}", [128, D], f32, kind="ExternalInput") for i in range(4)]
    gam_in = nc.dram_tensor("gamma_b", [128, D], f32, kind="ExternalInput")
    bet_in = nc.dram_tensor("beta_b", [128, D], f32, kind="ExternalInput")
    out = nc.dram_tensor("out", [NPC, D], f32, kind="ExternalOutput")

    Gelu = mybir.ActivationFunctionType.Gelu
    Sqrt = mybir.ActivationFunctionType.Sqrt
    Copy = mybir.ActivationFunctionType.Copy
    MUL = mybir.AluOpType.mult
    SUB = mybir.AluOpType.subtract
    ADD = mybir.AluOpType.add
    X = mybir.AxisListType.X

    with tile.TileContext(nc) as tc:
        with (
            tc.tile_pool(name="const", bufs=1) as constp,
            tc.tile_pool(name="meta", bufs=1) as metap,
            tc.tile_pool(name="xp", bufs=3) as xp,
            tc.tile_pool(name="msgp", bufs=2) as msgp,
            tc.tile_pool(name="sp", bufs=2) as sp,
            tc.tile_pool(name="aggp", bufs=4) as aggp,
            tc.tile_pool(name="hp", bufs=4) as hp,
            tc.tile_pool(name="lnp", bufs=4) as lnp,
            tc.tile_pool(name="ps1", bufs=3, space="PSUM") as ps1,
            tc.tile_pool(name="ps2", bufs=3, space="PSUM") as ps2,
            tc.tile_pool(name="dram", bufs=1, space="DRAM") as dram,
        ):
            # ---- constants / metadata into SBUF ----
            idx_sb = metap.tile([128, total_idxcols], i16)
            nc.sync.dma_start(idx_sb[:], idx_in[:])
            ns_sb = constp.tile([128, WINDOWS], f32)
            nc.sync.dma_start(ns_sb[:], ns_in[:])
            ndn_sb = constp.tile([128, WINDOWS], f32)
            nc.sync.dma_start(ndn_sb[:], ndn_in[:])
            gam_sb = constp.tile([128, D], f32)
            nc.sync.dma_start(gam_sb[:], gam_in[:])
            bet_sb = constp.tile([128, D], f32)
            nc.sync.dma_start(bet_sb[:], bet_in[:])
            w_sb = []
            bb_sb = []
            for i in range(4):
                wt = constp.tile([D, D], f16, name=f"w{i}_sb")
                nc.sync.dma_start(wt[:], w_in[i][:])
                w_sb.append(wt)
                bt = constp.tile([128, D], f32, name=f"bb{i}_sb")
                nc.sync.dma_start(bt[:], bb_in[i][:])
                bb_sb.append(bt)
            eps_t = constp.tile([128, 1], f32)
            nc.vector.memset(eps_t[:], 1e-5)

            # ---- DRAM h buffers ----
            h_shard = [
                dram.tile([NPC, D], f16, name=f"h_shard{l}") for l in range(4)
            ]
            h_bank = [
                [
                    dram.tile([BANK_ROWS, D], f16, addr_space="Shared",
                              name=f"h_bank{l}_{b}")
                    for b in range(BANKS)
                ]
                for l in range(4)
            ]
            rg = [list(range(NCORES))]

            def _ag(l, b):
                nc.gpsimd.collective_compute(
                    "AllGather", mybir.AluOpType.bypass, replica_groups=rg,
                    ins=[h_shard[l][b * CHUNK:(b + 1) * CHUNK, :]],
                    outs=[h_bank[l][b][:]],
                )

            def issue_ag(l, w):
                if w in AG_BOUNDARY:
                    b = AG_BOUNDARY.index(w)
                    if AG_LATE:
                        if b == BANKS - 1:
                            for bb in range(BANKS):
                                _ag(l, bb)
                    else:
                        _ag(l, b)

            # ---- prologue: h_shard0 = x * norm_src (cast fp16) ----
            for w in range(WINDOWS):
                rows = min(128, NPC - w * 128)
                xt = xp.tile([128, D], f32, tag="xt")
                nc.sync.dma_start(xt[:rows], x_in[w * 128:w * 128 + rows, :])
                ht = xp.tile([128, D], f16, tag="ht0")
                nc.scalar.activation(out=ht[:], in_=xt[:], func=Copy,
                                     scale=ns_sb[:, w:w + 1])
                nc.sync.dma_start(h_shard[0][w * 128:w * 128 + rows, :], ht[:rows])
                issue_ag(0, w)

            # ---- layers ----
            for l in range(dbg_layers):
                for si, k in enumerate(sizes):
                    # batched gathers per bank covering k windows, in chunks
                    # of <=8 subtiles (1024 idx: the SWDGE per-op ceiling)
                    msg_t = []
                    for b in range(BANKS):
                        msg = msgp.tile([128, K_SS * S * D], f16, tag=f"msg{b}")
                        nsub_all = k * S
                        c0 = 0
                        while c0 < nsub_all:
                            nsub_c = min(8, nsub_all - c0)
                            nidx = nsub_c * 128
                            nc.gpsimd.dma_gather(
                                msg[:, c0 * D:(c0 + nsub_c) * D].rearrange(
                                    "p (k d) -> p k d", d=D),
                                h_bank[l][b][:],
                                idx_sb[:, off_idx[si][b] + c0 * 8:
                                       off_idx[si][b] + c0 * 8 + nidx // 16],
                                nidx, nidx, D,
                                queue_num=b,
                            )
                            c0 += nsub_c
                        msg_t.append(msg)
                    # one-hot slab for the whole superstep
                    nsub_ss = k * BANKS * S
                    s_run = sp.tile([128, K_SS * BANKS * S * D], f8, tag="s")
                    nc.sync.dma_start(
                        s_run[:, :nsub_ss * D],
                        s8_in[:, off_sub[si] * D:(off_sub[si] + nsub_ss) * D],
                    )
                    for wl in range(k):
                        w = si * K_SS + wl
                        rows = min(128, NPC - w * 128)
                        psum1 = ps1.tile([128, 128], f32, tag="psum1")
                        n_tot = BANKS * S
                        mi = 0
                        for b in range(BANKS):
                            for s in range(S):
                                nc.tensor.matmul(
                                    psum1[:],
                                    lhsT=msg_t[b][:, (wl * S + s) * D:(wl * S + s + 1) * D],
                                    rhs=s_run[:, ((wl * BANKS + b) * S + s) * D:
                                              ((wl * BANKS + b) * S + s + 1) * D],
                                    start=(mi == 0), stop=(mi == n_tot - 1),
                                )
                                mi += 1
                        # dense: z[dst, of] = aggT.T @ W
                        aggT = aggp.tile([128, 128], f16, tag="aggT")
                        nc.scalar.copy(out=aggT[:], in_=psum1[:])
                        psum2 = ps2.tile([128, 128], f32, tag="psum2")
                        nc.tensor.matmul(psum2[:], lhsT=aggT[:], rhs=w_sb[l][:],
                                         start=True, stop=True)
                        # t2 = norm_dst * z + b  (fused on DVE)
                        t2 = hp.tile([128, D], f32, tag="t2")
                        nc.vector.scalar_tensor_tensor(
                            out=t2[:], in0=psum2[:], scalar=ndn_sb[:, w:w + 1],
                            in1=bb_sb[l][:], op0=MUL, op1=ADD,
                        )
                        if l < dbg_layers - 1:
                            g32 = hp.tile([128, D], f32, tag="g32")
                            nc.scalar.activation(out=g32[:], in_=t2[:], func=Gelu)
                            h16 = hp.tile([128, D], f16, tag="h16")
                            nc.vector.tensor_scalar(
                                out=h16[:], in0=g32[:],
                                scalar1=ns_sb[:, w:w + 1], scalar2=None, op0=MUL,
                            )
                            nc.sync.dma_start(
                                h_shard[l + 1][w * 128:w * 128 + rows, :],
                                h16[:rows],
                            )
                            issue_ag(l + 1, w)
                        else:
                            # LayerNorm over features
                            s1 = lnp.tile([128, 1], f32, tag="s1")
                            nc.vector.reduce_sum(s1[:], t2[:], axis=X)
                            mu = lnp.tile([128, 1], f32, tag="mu")
                            nc.scalar.mul(out=mu[:], in_=s1[:], mul=1.0 / D)
                            cent = lnp.tile([128, D], f32, tag="cent")
                            nc.vector.tensor_scalar(
                                out=cent[:], in0=t2[:], scalar1=mu[:],
                                scalar2=None, op0=SUB,
                            )
                            sq = lnp.tile([128, D], f32, tag="sq")
                            nc.vector.tensor_tensor(out=sq[:], in0=cent[:],
                                                    in1=cent[:], op=MUL)
                            vs = lnp.tile([128, 1], f32, tag="vs")
                            nc.vector.reduce_sum(vs[:], sq[:], axis=X)
                            std = lnp.tile([128, 1], f32, tag="std")
                            nc.scalar.activation(out=std[:], in_=vs[:], func=Sqrt,
                                                 scale=1.0 / D, bias=eps_t[:])
                            rstd = lnp.tile([128, 1], f32, tag="rstd")
                            nc.vector.reciprocal(out=rstd[:], in_=std[:])
                            t1 = lnp.tile([128, D], f32, tag="t1")
                            nc.vector.tensor_scalar(out=t1[:], in0=cent[:],
                                                    scalar1=rstd[:], scalar2=None,
                                                    op0=MUL)
                            t4 = lnp.tile([128, D], f32, tag="t4")
                            nc.vector.tensor_tensor(out=t4[:], in0=t1[:],
                                                    in1=gam_sb[:], op=MUL)
                            t5 = lnp.tile([128, D], f32, tag="t5")
                            nc.vector.tensor_tensor(out=t5[:], in0=t4[:],
                                                    in1=bet_sb[:], op=ADD)
                            nc.sync.dma_start(
                                out[w * 128:w * 128 + rows, :], t5[:rows]
                            )
    nc.compile()
    return nc


def kernel(**inputs):
    global LAST_EXEC_NS
    from concourse.bass_utils import run_bass_kernel_spmd

    x = np.asarray(inputs["x"], np.float32)
    src = inputs["src"]
    dst = inputs["dst"]

    key = "prog"
    if key not in _CACHE:
        S, off_idx, off_sub, tic, tsc, per_core, ns_tiles, ndn_tiles = \
            _prep_graph(src, dst)
        nc = _build_program(S, off_idx, off_sub, tic, tsc)
        _CACHE[key] = (nc, per_core, ns_tiles, ndn_tiles)
    nc, per_core, ns_tiles, ndn_tiles = _CACHE[key]

    gamma = np.asarray(inputs["gamma"], np.float32).reshape(1, D)
    beta = np.asarray(inputs["beta"], np.float32).reshape(1, D)
    gamma_b = np.repeat(gamma, 128, axis=0)
    beta_b = np.repeat(beta, 128, axis=0)

    in_maps = []
    for c in range(NCORES):
        idx16, s8 = per_core[c]
        m = {
            "x": np.ascontiguousarray(x[c * NPC:(c + 1) * NPC]),
            "idx16": idx16,
            "s8": s8,
            "ns": ns_tiles[c],
            "ndn": ndn_tiles[c],
            "gamma_b": gamma_b,
            "beta_b": beta_b,
        }
        for i in range(4):
            m[f"W{i+1}"] = np.asarray(inputs[f"W{i+1}"], np.float32).astype(np.float16)
            bb = np.asarray(inputs[f"b{i+1}"], np.float32).reshape(1, D)
            m[f"bb{i+1}"] = np.repeat(bb, 128, axis=0)
        in_maps.append(m)

    if TRACE:
        _install_ntff_hook()
    res = run_bass_kernel_spmd(
        nc, in_maps, core_ids=list(range(NCORES)), trace=TRACE
    )
    LAST_EXEC_NS = res.exec_time_ns
    return np.concatenate(
        [res.results[c]["out"] for c in range(NCORES)], axis=0
    ).astype(np.float32)


# revision 16
# speedup vs baseline: 1.1334x; 1.1334x over previous
"""4-layer GCN (EnhancedGCN) on 8 Trainium2 NeuronCores.

Strategy (node/graph parallel):
  - Nodes sharded 12500/core across 8 cores; edges assigned to the core
    owning their dst node.
  - Per layer, h (pre-scaled by norm_src, fp16) is replicated via FOUR
    chunked AllGathers (chunk b = rows [b*3125,(b+1)*3125) of every core's
    shard -> one 25000-row "bank" tensor); chunk AGs are issued as soon as
    the producing windows finish, so they overlap tail-window compute.
  - Each core gathers the src rows for its edges with dma_gather, one call
    per (superstep of 4 dst windows) x bank (2560 idx / call) to amortize
    the ~1us SWDGE fixed cost; indices are sorted ascending within each
    (window,bank) group for DRAM page locality.
  - Aggregation per 128-node dst window: one-hot matmuls (fp16 msg x fp8
    one-hot slab streamed from HBM) accumulated in PSUM, then the dense W
    matmul; norm_dst*z + b on DVE; GELU + PSUM copy on the scalar (ACT)
    engine to keep DVE slack; final LayerNorm on DVE.
  - Graph preprocessing (degree norms, edge grouping with a uniform
    subtile count, padding, gather index layout) happens on host once; the
    compiled program is shared by all 8 cores (SPMD).
"""

import sys
import types

import numpy as np

N_NODES = 100000
N_EDGES = 1600000
D = 128
NCORES = 8
NPC = N_NODES // NCORES            # 12500 nodes per core
WINDOWS = (NPC + 127) // 128       # 98 dst windows per core (last has 84 rows)
BANKS = 4
CHUNK = NPC // BANKS               # 3125 rows per AG chunk per core
BANK_ROWS = CHUNK * NCORES         # 25000 rows per bank tensor (int16-safe)
import os as _os
K_SS = int(_os.environ.get("KSS", "4"))  # dst windows per superstep (gather batch)
AG_LATE = _os.environ.get("AGLATE", "") != ""  # issue chunk AGs only at layer end
FP8 = _os.environ.get("FP8", "1") != ""   # fp8 h replicas (128B gather rows)
PAD_DLOC = 999.0

TRACE = False
LAST_EXEC_NS = None

_CACHE = {}


def _install_ntff_hook():
    if "antenv.axon_hooks" in sys.modules:
        return
    mod = types.ModuleType("antenv.axon_hooks")
    _hook = [None]
    mod.set_axon_ntff_profile_hook = lambda h: _hook.__setitem__(0, h)
    mod.get_axon_ntff_profile_hook = lambda: _hook[0]
    sys.modules["antenv.axon_hooks"] = mod
    import antenv

    antenv.axon_hooks = mod
    try:
        from trn_agent_boot.trn_boot import _ntff_profile_via_ctypes

        mod.set_axon_ntff_profile_hook(
            _ntff_profile_via_ctypes("/opt/axon/libaxon_pjrt.so")
        )
    except Exception:
        pass


def _superstep_sizes():
    sizes = []
    w = 0
    while w < WINDOWS:
        k = min(K_SS, WINDOWS - w)
        sizes.append(k)
        w += k
    return sizes


def _prep_graph(src, dst):
    """Host-side graph preprocessing shared by all layers."""
    src = np.asarray(src).astype(np.int64).ravel()
    dst = np.asarray(dst).astype(np.int64).ravel()

    deg_src = np.bincount(src, minlength=N_NODES).astype(np.float64)
    deg_dst = np.bincount(dst, minlength=N_NODES).astype(np.float64)
    norm_src = np.clip(deg_src, 1.0, None) ** -0.5
    norm_dst = np.clip(deg_dst, 1.0, None) ** -0.5

    # src node -> (bank, row-in-bank) under the chunked-AG layout:
    # bank b holds rows [c*3125+(r%3125)] = AllGather of every core's slice b
    s_core = src // NPC
    s_rem = src % NPC
    s_bank = s_rem // CHUNK
    s_row = s_core * CHUNK + (s_rem % CHUNK)   # 0..24999

    core = dst // NPC
    w = (dst % NPC) // 128
    group = (core * WINDOWS + w) * BANKS + s_bank
    order = np.argsort(group, kind="stable")
    g_sorted = group[order]
    row_sorted = s_row[order]
    dst_sorted = dst[order]

    n_groups = NCORES * WINDOWS * BANKS
    counts = np.bincount(g_sorted, minlength=n_groups)
    starts = np.zeros(n_groups + 1, np.int64)
    np.cumsum(counts, out=starts[1:])

    S = int(np.ceil(counts.max() / 128.0))   # uniform subtiles per (w,b)
    CAP = S * 128

    sizes = _superstep_sizes()
    # idx col offsets: [ss][b] -> start col in idx16 (16 idx per col)
    off_idx = []
    icol = 0
    for k in sizes:
        row_b = []
        for b in range(BANKS):
            row_b.append(icol)
            icol += k * CAP // 16
        off_idx.append(row_b)
    total_idxcols = icol
    # s8 col offsets: [ss] -> start sub; subs within ss: (wl*BANKS+b)*S+s
    off_sub = []
    scol = 0
    for k in sizes:
        off_sub.append(scol)
        scol += k * BANKS * S
    total_subs = scol

    per_core = []
    for c in range(NCORES):
        idx16 = np.zeros((128, total_idxcols), np.int16)
        dloc = np.full((128, total_subs), PAD_DLOC, np.float16)
        for si, k in enumerate(sizes):
            for b in range(BANKS):
                icol0 = off_idx[si][b]
                for wl in range(k):
                    wi = si * K_SS + wl
                    gidx = (c * WINDOWS + wi) * BANKS + b
                    s0, s1 = starts[gidx], starts[gidx + 1]
                    n_e = s1 - s0
                    loc = np.zeros(CAP, np.int64)
                    dl = np.full(CAP, PAD_DLOC, np.float64)
                    # ascending src rows within the group: DRAM locality
                    loc[:n_e] = row_sorted[s0:s1]
                    dl[:n_e] = (dst_sorted[s0:s1] % NPC) - wi * 128
                    # idx layout: index i -> partition i%16, col i//16,
                    # replicated across the 8 partition stripes
                    stripe = loc.reshape(CAP // 16, 16).T.astype(np.int16)
                    cc0 = icol0 + wl * (CAP // 16)
                    for st in range(8):
                        idx16[16 * st:16 * st + 16, cc0:cc0 + CAP // 16] = stripe
                    # subtile layout: edge i -> partition i%128, subtile i//128
                    sub0 = off_sub[si] + (wl * BANKS + b) * S
                    dloc[:, sub0:sub0 + S] = dl.reshape(S, 128).T.astype(np.float16)
        onehot = (
            dloc[:, :, None] == np.arange(128, dtype=np.float16)[None, None, :]
        )
        import ml_dtypes
        s8 = onehot.astype(ml_dtypes.float8_e4m3).reshape(128, total_subs * 128)
        per_core.append((idx16, s8))

    def node_tile(vec, c):
        full = np.zeros(WINDOWS * 128, np.float32)
        full[:NPC] = vec[c * NPC:(c + 1) * NPC].astype(np.float32)
        return full.reshape(WINDOWS, 128).T.copy()

    ns_tiles = [node_tile(norm_src, c) for c in range(NCORES)]
    ndn_tiles = [node_tile(norm_dst, c) for c in range(NCORES)]

    return S, off_idx, off_sub, total_idxcols, total_subs, per_core, \
        ns_tiles, ndn_tiles


# AG chunk b is complete once windows 0..AG_BOUNDARY[b] have been written.
AG_BOUNDARY = [
    ((b + 1) * CHUNK + 127) // 128 - 1 for b in range(BANKS)
]  # [24, 48, 73, 97]


def _dma_gather_raw(nc, mybir, out_ap, in_ap, idxs_ap, num_idxs,
                    elem_size, elem_step, queue_num):
    """dma_gather of elem_size-element rows from a table with a wider
    (256B-aligned) row stride: the public helper asserts elem_size itself
    is 256B-aligned, which only the transpose path needs."""
    gp = nc.gpsimd
    gp._assert_queue_num(queue_num)
    stride_bytes = elem_step * mybir.dt.size(in_ap.dtype)
    assert stride_bytes % 256 == 0
    _in_ap = gp.lower_ap_dma(in_ap, for_custom_bir_dma=True)
    _idxs_ap = gp.lower_ap(idxs_ap)
    _out_ap = gp.lower_ap(out_ap)
    return gp.add_instruction(
        mybir.InstDMAGatherAnt(
            name=nc.get_next_instruction_name(),
            ins=[*_in_ap, _idxs_ap, gp.lower_val_access(gp.to_reg(num_idxs))],
            outs=[_out_ap],
            transpose=False,
            num_idxs=num_idxs,
            elem_size=elem_size,
            stride_bytes_256=stride_bytes // 256,
            gen_mode=0,
            single_packet=True,
            queue_num=queue_num,
            sbuf_tokens_per_rank=0,
            sbuf_free_dim_per_rank=0,
            sbuf_free_dim_pad_per_rank=0,
            sbuf_byte_offset=0,
        )
    )


def _build_program(S, off_idx, off_sub, total_idxcols, total_subs):
    import os

    import concourse.bacc as bacc
    import concourse.mybir as mybir
    import concourse.tile as tile

    dbg_layers = int(os.environ.get("DBG_LAYERS", "4"))
    CAP = S * 128
    sizes = _superstep_sizes()

    nc = bacc.Bacc(
        "TRN2",
        target_bir_lowering=False,
        debug=False,
        enable_asserts=False,
        num_devices=NCORES,
        num_swdge_queues=BANKS,
        # headroom over the default 16 KiB carveout (1024 descriptors) so
        # back-to-back 1024-idx gathers on one queue never wait for reclaim
        dynamic_dma_scratch_size=49152,
    )
    f32, f16, i16 = mybir.dt.float32, mybir.dt.float16, mybir.dt.int16
    f8 = mybir.dt.float8e4

    x_in = nc.dram_tensor("x", [NPC, D], f32, kind="ExternalInput")
    idx_in = nc.dram_tensor("idx16", [128, total_idxcols], i16, kind="ExternalInput")
    s8_in = nc.dram_tensor("s8", [128, total_subs * D], f8, kind="ExternalInput")
    ns_in = nc.dram_tensor("ns", [128, WINDOWS], f32, kind="ExternalInput")
    ndn_in = nc.dram_tensor("ndn", [128, WINDOWS], f32, kind="ExternalInput")
    w_in = [nc.dram_tensor(f"W{i+1}", [D, D], f16, kind="ExternalInput") for i in range(4)]
    bb_in = [nc.dram_tensor(f"bb{i+1}", [128, D], f32, kind="ExternalInput") for i in range(4)]
    gam_in = nc.dram_tensor("gamma_b", [128, D], f32, kind="ExternalInput")
    bet_in = nc.dram_tensor("beta_b", [128, D], f32, kind="ExternalInput")
    out = nc.dram_tensor("out", [NPC, D], f32, kind="ExternalOutput")

    Gelu = mybir.ActivationFunctionType.Gelu
    Sqrt = mybir.ActivationFunctionType.Sqrt
    Copy = mybir.ActivationFunctionType.Copy
    Square = mybir.ActivationFunctionType.Square
    MUL = mybir.AluOpType.mult
    SUB = mybir.AluOpType.subtract
    ADD = mybir.AluOpType.add
    X = mybir.AxisListType.X

    with tile.TileContext(nc) as tc:
        with (
            tc.tile_pool(name="const", bufs=1) as constp,
            tc.tile_pool(name="meta", bufs=1) as metap,
            tc.tile_pool(name="xp", bufs=3) as xp,
            tc.tile_pool(name="msgp", bufs=2) as msgp,
            tc.tile_pool(name="sp", bufs=2) as sp,
            tc.tile_pool(name="aggp", bufs=4) as aggp,
            tc.tile_pool(name="hp", bufs=4) as hp,
            tc.tile_pool(name="lnp", bufs=4) as lnp,
            tc.tile_pool(name="ps1", bufs=3, space="PSUM") as ps1,
            tc.tile_pool(name="ps2", bufs=3, space="PSUM") as ps2,
            tc.tile_pool(name="dram", bufs=1, space="DRAM") as dram,
        ):
            # ---- constants / metadata into SBUF ----
            idx_sb = metap.tile([128, total_idxcols], i16)
            nc.sync.dma_start(idx_sb[:], idx_in[:])
            ns_sb = constp.tile([128, WINDOWS], f32)
            nc.sync.dma_start(ns_sb[:], ns_in[:])
            ndn_sb = constp.tile([128, WINDOWS], f32)
            nc.sync.dma_start(ndn_sb[:], ndn_in[:])
            gam_sb = constp.tile([128, D], f32)
            nc.sync.dma_start(gam_sb[:], gam_in[:])
            bet_sb = constp.tile([128, D], f32)
            nc.sync.dma_start(bet_sb[:], bet_in[:])
            w_sb = []
            bb_sb = []
            for i in range(4):
                wt = constp.tile([D, D], f16, name=f"w{i}_sb")
                nc.sync.dma_start(wt[:], w_in[i][:])
                w_sb.append(wt)
                bt = constp.tile([128, D], f32, name=f"bb{i}_sb")
                nc.sync.dma_start(bt[:], bb_in[i][:])
                bb_sb.append(bt)
            eps_t = constp.tile([128, 1], f32)
            nc.vector.memset(eps_t[:], 1e-5)

            # ---- DRAM h buffers ----
            h_shard = [
                dram.tile([NPC, D], f16, name=f"h_shard{l}") for l in range(4)
            ]
            h_bank = [
                [
                    dram.tile([BANK_ROWS, D], f16, addr_space="Shared",
                              name=f"h_bank{l}_{b}")
                    for b in range(BANKS)
                ]
                for l in range(4)
            ]
            rg = [list(range(NCORES))]

            def _ag(l, b):
                nc.gpsimd.collective_compute(
                    "AllGather", mybir.AluOpType.bypass, replica_groups=rg,
                    ins=[h_shard[l][b * CHUNK:(b + 1) * CHUNK, :]],
                    outs=[h_bank[l][b][:]],
                )

            def issue_ag(l, w):
                if w in AG_BOUNDARY:
                    b = AG_BOUNDARY.index(w)
                    if AG_LATE:
                        if b == BANKS - 1:
                            for bb in range(BANKS):
                                _ag(l, bb)
                    else:
                        _ag(l, b)

            # ---- prologue: h_shard0 = x * norm_src (cast fp16) ----
            for w in range(WINDOWS):
                rows = min(128, NPC - w * 128)
                xt = xp.tile([128, D], f32, tag="xt")
                nc.sync.dma_start(xt[:rows], x_in[w * 128:w * 128 + rows, :])
                ht = xp.tile([128, D], f16, tag="ht0")
                nc.scalar.activation(out=ht[:], in_=xt[:], func=Copy,
                                     scale=ns_sb[:, w:w + 1])
                nc.sync.dma_start(h_shard[0][w * 128:w * 128 + rows, :], ht[:rows])
                issue_ag(0, w)

            # ---- layers ----
            for l in range(dbg_layers):
                for si, k in enumerate(sizes):
                    # batched gathers per bank covering k windows, in chunks
                    # of <=8 subtiles (1024 idx: the SWDGE per-op ceiling);
                    # chunks interleave across banks so Q7 never head-of-line
                    # blocks on one queue's ring space
                    msg_t = [
                        msgp.tile([128, K_SS * S * D], f16, tag=f"msg{b}",
                                  name=f"msg{b}")
                        for b in range(BANKS)
                    ]
                    nsub_all = k * S
                    c0 = 0
                    while c0 < nsub_all:
                        nsub_c = min(8, nsub_all - c0)
                        nidx = nsub_c * 128
                        for b in range(BANKS):
                            nc.gpsimd.dma_gather(
                                msg_t[b][:, c0 * D:(c0 + nsub_c) * D].rearrange(
                                    "p (k d) -> p k d", d=D),
                                h_bank[l][b][:],
                                idx_sb[:, off_idx[si][b] + c0 * 8:
                                       off_idx[si][b] + c0 * 8 + nidx // 16],
                                nidx, nidx, D,
                                queue_num=b,
                            )
                        c0 += nsub_c
                    # one-hot slab for the whole superstep
                    nsub_ss = k * BANKS * S
                    s_run = sp.tile([128, K_SS * BANKS * S * D], f8, tag="s")
                    nc.sync.dma_start(
                        s_run[:, :nsub_ss * D],
                        s8_in[:, off_sub[si] * D:(off_sub[si] + nsub_ss) * D],
                    )
                    for wl in range(k):
                        w = si * K_SS + wl
                        rows = min(128, NPC - w * 128)
                        psum1 = ps1.tile([128, 128], f32, tag="psum1")
                        n_tot = BANKS * S
                        mi = 0
                        for b in range(BANKS):
                            for s in range(S):
                                nc.tensor.matmul(
                                    psum1[:],
                                    lhsT=msg_t[b][:, (wl * S + s) * D:(wl * S + s + 1) * D],
                                    rhs=s_run[:, ((wl * BANKS + b) * S + s) * D:
                                              ((wl * BANKS + b) * S + s + 1) * D],
                                    start=(mi == 0), stop=(mi == n_tot - 1),
                                )
                                mi += 1
                        # dense: z[dst, of] = aggT.T @ W
                        aggT = aggp.tile([128, 128], f16, tag="aggT")
                        nc.scalar.copy(out=aggT[:], in_=psum1[:])
                        psum2 = ps2.tile([128, 128], f32, tag="psum2")
                        nc.tensor.matmul(psum2[:], lhsT=aggT[:], rhs=w_sb[l][:],
                                         start=True, stop=True)
                        # t2 = norm_dst * z + b  (fused on DVE);
                        # last layer also accumulates sum(t2) for LayerNorm
                        t2 = hp.tile([128, D], f32, tag="t2")
                        last = l == dbg_layers - 1
                        s1 = (lnp.tile([128, 1], f32, tag="s1", name="s1")
                              if last else None)
                        nc.vector.scalar_tensor_tensor(
                            out=t2[:], in0=psum2[:], scalar=ndn_sb[:, w:w + 1],
                            in1=bb_sb[l][:], op0=MUL, op1=ADD,
                            accum_out=s1[:] if last else None,
                        )
                        if l < dbg_layers - 1:
                            g32 = hp.tile([128, D], f32, tag="g32")
                            nc.scalar.activation(out=g32[:], in_=t2[:], func=Gelu)
                            h16 = hp.tile([128, D], f16, tag="h16")
                            nc.vector.tensor_scalar(
                                out=h16[:], in0=g32[:],
                                scalar1=ns_sb[:, w:w + 1], scalar2=None, op0=MUL,
                            )
                            nc.sync.dma_start(
                                h_shard[l + 1][w * 128:w * 128 + rows, :],
                                h16[:rows],
                            )
                            issue_ag(l + 1, w)
                        else:
                            # LayerNorm over features: Σt2 came free from the
                            # STT accum; Σt2² via ACT Square accum
                            sq = lnp.tile([128, D], f32, tag="sq")
                            vs = lnp.tile([128, 1], f32, tag="vs")
                            nc.scalar.activation(out=sq[:], in_=t2[:],
                                                 func=Square,
                                                 accum_out=vs[:])
                            mu = lnp.tile([128, 1], f32, tag="mu")
                            nc.scalar.mul(out=mu[:], in_=s1[:], mul=1.0 / D)
                            mu2 = lnp.tile([128, 1], f32, tag="mu2")
                            nc.vector.tensor_tensor(out=mu2[:], in0=mu[:],
                                                    in1=mu[:], op=MUL)
                            bia = lnp.tile([128, 1], f32, tag="bia")
                            nc.vector.tensor_tensor(out=bia[:], in0=eps_t[:],
                                                    in1=mu2[:], op=SUB)
                            # std = sqrt(E[t2^2] - mu^2 + eps)
                            std = lnp.tile([128, 1], f32, tag="std")
                            nc.scalar.activation(out=std[:], in_=vs[:], func=Sqrt,
                                                 scale=1.0 / D, bias=bia[:])
                            rstd = lnp.tile([128, 1], f32, tag="rstd")
                            nc.vector.reciprocal(out=rstd[:], in_=std[:])
                            cent = lnp.tile([128, D], f32, tag="cent")
                            nc.vector.tensor_scalar(
                                out=cent[:], in0=t2[:], scalar1=mu[:],
                                scalar2=None, op0=SUB,
                            )
                            t4 = lnp.tile([128, D], f32, tag="t4")
                            nc.vector.scalar_tensor_tensor(
                                out=t4[:], in0=cent[:], scalar=rstd[:],
                                in1=gam_sb[:], op0=MUL, op1=MUL,
                            )
                            t5 = lnp.tile([128, D], f32, tag="t5")
                            nc.vector.tensor_tensor(out=t5[:], in0=t4[:],
                                                    in1=bet_sb[:], op=ADD)
                            nc.sync.dma_start(
                                out[w * 128:w * 128 + rows, :], t5[:rows]
                            )
    nc.compile()
    return nc


def kernel(**inputs):
    global LAST_EXEC_NS
    from concourse.bass_utils import run_bass_kernel_spmd

    x = np.asarray(inputs["x"], np.float32)
    src = inputs["src"]
    dst = inputs["dst"]

    key = "prog"
    if key not in _CACHE:
        S, off_idx, off_sub, tic, tsc, per_core, ns_tiles, ndn_tiles = \
            _prep_graph(src, dst)
        nc = _build_program(S, off_idx, off_sub, tic, tsc)
        _CACHE[key] = (nc, per_core, ns_tiles, ndn_tiles)
    nc, per_core, ns_tiles, ndn_tiles = _CACHE[key]

    gamma = np.asarray(inputs["gamma"], np.float32).reshape(1, D)
    beta = np.asarray(inputs["beta"], np.float32).reshape(1, D)
    gamma_b = np.repeat(gamma, 128, axis=0)
    beta_b = np.repeat(beta, 128, axis=0)

    in_maps = []
    for c in range(NCORES):
        idx16, s8 = per_core[c]
        m = {
            "x": np.ascontiguousarray(x[c * NPC:(c + 1) * NPC]),
            "idx16": idx16,
            "s8": s8,
            "ns": ns_tiles[c],
            "ndn": ndn_tiles[c],
            "gamma_b": gamma_b,
            "beta_b": beta_b,
        }
        for i in range(4):
            m[f"W{i+1}"] = np.asarray(inputs[f"W{i+1}"], np.float32).astype(np.float16)
            bb = np.asarray(inputs[f"b{i+1}"], np.float32).reshape(1, D)
            m[f"bb{i+1}"] = np.repeat(bb, 128, axis=0)
        in_maps.append(m)

    if TRACE:
        _install_ntff_hook()
    res = run_bass_kernel_spmd(
        nc, in_maps, core_ids=list(range(NCORES)), trace=TRACE
    )
    LAST_EXEC_NS = res.exec_time_ns
    return np.concatenate(
        [res.results[c]["out"] for c in range(NCORES)], axis=0
    ).astype(np.float32)


# revision 21
# speedup vs baseline: 1.2170x; 1.0738x over previous
"""4-layer GCN (EnhancedGCN) on 8 Trainium2 NeuronCores.

Strategy (node/graph parallel):
  - Nodes sharded 12500/core across 8 cores; edges assigned to the core
    owning their dst node.
  - Per layer, h (pre-scaled by norm_src, fp16) is replicated via FOUR
    chunked AllGathers (chunk b = rows [b*3125,(b+1)*3125) of every core's
    shard -> one 25000-row "bank" tensor); chunk AGs are issued as soon as
    the producing windows finish, so they overlap tail-window compute.
  - Each core gathers the src rows for its edges with dma_gather, one call
    per (superstep of 4 dst windows) x bank (2560 idx / call) to amortize
    the ~1us SWDGE fixed cost; indices are sorted ascending within each
    (window,bank) group for DRAM page locality.
  - Aggregation per 128-node dst window: one-hot matmuls (fp16 msg x fp8
    one-hot slab streamed from HBM) accumulated in PSUM, then the dense W
    matmul; norm_dst*z + b on DVE; GELU + PSUM copy on the scalar (ACT)
    engine to keep DVE slack; final LayerNorm on DVE.
  - Graph preprocessing (degree norms, edge grouping with a uniform
    subtile count, padding, gather index layout) happens on host once; the
    compiled program is shared by all 8 cores (SPMD).
"""

import sys
import types

import numpy as np

N_NODES = 100000
N_EDGES = 1600000
D = 128
NCORES = 8
NPC = N_NODES // NCORES            # 12500 nodes per core
WINDOWS = (NPC + 127) // 128       # 98 dst windows per core (last has 84 rows)
BANKS = 4
CHUNK = NPC // BANKS               # 3125 rows per AG chunk per core
BANK_ROWS = CHUNK * NCORES         # 25000 rows per bank tensor (int16-safe)
import os as _os
K_SS = int(_os.environ.get("KSS", "4"))  # dst windows per superstep (gather batch)
AG_LATE = _os.environ.get("AGLATE", "") != ""  # issue chunk AGs only at layer end
FP8 = _os.environ.get("FP8", "1") != ""   # fp8 h replicas (128B gather rows)
PAD_DLOC = 999.0

TRACE = False
LAST_EXEC_NS = None

_CACHE = {}


def _install_ntff_hook():
    if "antenv.axon_hooks" in sys.modules:
        return
    mod = types.ModuleType("antenv.axon_hooks")
    _hook = [None]
    mod.set_axon_ntff_profile_hook = lambda h: _hook.__setitem__(0, h)
    mod.get_axon_ntff_profile_hook = lambda: _hook[0]
    sys.modules["antenv.axon_hooks"] = mod
    import antenv

    antenv.axon_hooks = mod
    try:
        from trn_agent_boot.trn_boot import _ntff_profile_via_ctypes

        mod.set_axon_ntff_profile_hook(
            _ntff_profile_via_ctypes("/opt/axon/libaxon_pjrt.so")
        )
    except Exception:
        pass


def _superstep_sizes():
    sizes = []
    w = 0
    while w < WINDOWS:
        k = min(K_SS, WINDOWS - w)
        sizes.append(k)
        w += k
    return sizes


def _prep_graph(src, dst):
    """Host-side graph preprocessing shared by all layers."""
    src = np.asarray(src).astype(np.int64).ravel()
    dst = np.asarray(dst).astype(np.int64).ravel()

    deg_src = np.bincount(src, minlength=N_NODES).astype(np.float64)
    deg_dst = np.bincount(dst, minlength=N_NODES).astype(np.float64)
    norm_src = np.clip(deg_src, 1.0, None) ** -0.5
    norm_dst = np.clip(deg_dst, 1.0, None) ** -0.5

    # src node -> (bank, row-in-bank) under the chunked-AG layout:
    # bank b holds rows [c*3125+(r%3125)] = AllGather of every core's slice b
    s_core = src // NPC
    s_rem = src % NPC
    s_bank = s_rem // CHUNK
    s_row = s_core * CHUNK + (s_rem % CHUNK)   # 0..24999

    core = dst // NPC
    w = (dst % NPC) // 128
    group = (core * WINDOWS + w) * BANKS + s_bank
    order = np.argsort(group, kind="stable")
    g_sorted = group[order]
    row_sorted = s_row[order]
    dst_sorted = dst[order]

    n_groups = NCORES * WINDOWS * BANKS
    counts = np.bincount(g_sorted, minlength=n_groups)
    starts = np.zeros(n_groups + 1, np.int64)
    np.cumsum(counts, out=starts[1:])

    S = int(np.ceil(counts.max() / 128.0))   # uniform subtiles per (w,b)
    CAP = S * 128

    sizes = _superstep_sizes()
    # idx col offsets: [ss][b] -> start col in idx16 (16 idx per col)
    off_idx = []
    icol = 0
    for k in sizes:
        row_b = []
        for b in range(BANKS):
            row_b.append(icol)
            icol += k * CAP // 16
        off_idx.append(row_b)
    total_idxcols = icol
    # s8 col offsets: [ss] -> start sub; subs within ss: (wl*BANKS+b)*S+s
    off_sub = []
    scol = 0
    for k in sizes:
        off_sub.append(scol)
        scol += k * BANKS * S
    total_subs = scol

    per_core = []
    for c in range(NCORES):
        idx16 = np.zeros((128, total_idxcols), np.int16)
        dloc = np.full((128, total_subs), PAD_DLOC, np.float16)
        for si, k in enumerate(sizes):
            for b in range(BANKS):
                icol0 = off_idx[si][b]
                for wl in range(k):
                    wi = si * K_SS + wl
                    gidx = (c * WINDOWS + wi) * BANKS + b
                    s0, s1 = starts[gidx], starts[gidx + 1]
                    n_e = s1 - s0
                    loc = np.zeros(CAP, np.int64)
                    dl = np.full(CAP, PAD_DLOC, np.float64)
                    # ascending src rows within the group: DRAM locality
                    loc[:n_e] = row_sorted[s0:s1]
                    dl[:n_e] = (dst_sorted[s0:s1] % NPC) - wi * 128
                    # idx layout: index i -> partition i%16, col i//16,
                    # replicated across the 8 partition stripes
                    stripe = loc.reshape(CAP // 16, 16).T.astype(np.int16)
                    cc0 = icol0 + wl * (CAP // 16)
                    for st in range(8):
                        idx16[16 * st:16 * st + 16, cc0:cc0 + CAP // 16] = stripe
                    # subtile layout: edge i -> partition i%128, subtile i//128
                    sub0 = off_sub[si] + (wl * BANKS + b) * S
                    dloc[:, sub0:sub0 + S] = dl.reshape(S, 128).T.astype(np.float16)
        onehot = (
            dloc[:, :, None] == np.arange(128, dtype=np.float16)[None, None, :]
        )
        import ml_dtypes
        s8 = onehot.astype(ml_dtypes.float8_e4m3).reshape(128, total_subs * 128)
        per_core.append((idx16, s8))

    def node_tile(vec, c):
        full = np.zeros(WINDOWS * 128, np.float32)
        full[:NPC] = vec[c * NPC:(c + 1) * NPC].astype(np.float32)
        return full.reshape(WINDOWS, 128).T.copy()

    ns_tiles = [node_tile(norm_src, c) for c in range(NCORES)]
    ndn_tiles = [node_tile(norm_dst, c) for c in range(NCORES)]

    return S, off_idx, off_sub, total_idxcols, total_subs, per_core, \
        ns_tiles, ndn_tiles


# AG chunk b is complete once windows 0..AG_BOUNDARY[b] have been written.
AG_BOUNDARY = [
    ((b + 1) * CHUNK + 127) // 128 - 1 for b in range(BANKS)
]  # [24, 48, 73, 97]


def _dma_gather_raw(nc, mybir, out_ap, in_ap, idxs_ap, num_idxs,
                    elem_size, elem_step, queue_num):
    """dma_gather of elem_size-element rows from a table with a wider
    (256B-aligned) row stride: the public helper asserts elem_size itself
    is 256B-aligned, which only the transpose path needs."""
    gp = nc.gpsimd
    gp._assert_queue_num(queue_num)
    stride_bytes = elem_step * mybir.dt.size(in_ap.dtype)
    assert stride_bytes % 256 == 0
    _in_ap = gp.lower_ap_dma(in_ap, for_custom_bir_dma=True)
    _idxs_ap = gp.lower_ap(idxs_ap)
    _out_ap = gp.lower_ap(out_ap)
    return gp.add_instruction(
        mybir.InstDMAGatherAnt(
            name=nc.get_next_instruction_name(),
            ins=[*_in_ap, _idxs_ap, gp.lower_val_access(gp.to_reg(num_idxs))],
            outs=[_out_ap],
            transpose=False,
            num_idxs=num_idxs,
            elem_size=elem_size,
            stride_bytes_256=stride_bytes // 256,
            gen_mode=0,
            single_packet=True,
            queue_num=queue_num,
            sbuf_tokens_per_rank=0,
            sbuf_free_dim_per_rank=0,
            sbuf_free_dim_pad_per_rank=0,
            sbuf_byte_offset=0,
        )
    )


def _build_program(S, off_idx, off_sub, total_idxcols, total_subs):
    import os

    import concourse.bacc as bacc
    import concourse.mybir as mybir
    import concourse.tile as tile

    dbg_layers = int(os.environ.get("DBG_LAYERS", "4"))
    CAP = S * 128
    sizes = _superstep_sizes()

    nc = bacc.Bacc(
        "TRN2",
        target_bir_lowering=False,
        debug=False,
        enable_asserts=False,
        num_devices=NCORES,
        num_swdge_queues=BANKS,
        # headroom over the default 16 KiB carveout (1024 descriptors) so
        # back-to-back 1024-idx gathers on one queue never wait for reclaim
        dynamic_dma_scratch_size=49152,
    )
    f32, f16, i16 = mybir.dt.float32, mybir.dt.float16, mybir.dt.int16
    f8 = mybir.dt.float8e4

    x_in = nc.dram_tensor("x", [NPC, D], f32, kind="ExternalInput")
    idx_in = nc.dram_tensor("idx16", [128, total_idxcols], i16, kind="ExternalInput")
    s8_in = nc.dram_tensor("s8", [128, total_subs * D], f8, kind="ExternalInput")
    ns_in = nc.dram_tensor("ns", [128, WINDOWS], f32, kind="ExternalInput")
    ndn_in = nc.dram_tensor("ndn", [128, WINDOWS], f32, kind="ExternalInput")
    w_in = [nc.dram_tensor(f"W{i+1}", [D, D], f16, kind="ExternalInput") for i in range(4)]
    bb_in = [nc.dram_tensor(f"bb{i+1}", [128, D], f32, kind="ExternalInput") for i in range(4)]
    gam_in = nc.dram_tensor("gamma_b", [128, D], f32, kind="ExternalInput")
    bet_in = nc.dram_tensor("beta_b", [128, D], f32, kind="ExternalInput")
    out = nc.dram_tensor("out", [NPC, D], f32, kind="ExternalOutput")

    Gelu = mybir.ActivationFunctionType.Gelu
    Sqrt = mybir.ActivationFunctionType.Sqrt
    Copy = mybir.ActivationFunctionType.Copy
    Square = mybir.ActivationFunctionType.Square
    fh = f8 if FP8 else f16          # replicated-h dtype
    ROWB = 2 * D if FP8 else D       # h row width in elements (256B stride)
    MUL = mybir.AluOpType.mult
    SUB = mybir.AluOpType.subtract
    ADD = mybir.AluOpType.add
    X = mybir.AxisListType.X

    with tile.TileContext(nc) as tc:
        with (
            tc.tile_pool(name="const", bufs=1) as constp,
            tc.tile_pool(name="meta", bufs=1) as metap,
            tc.tile_pool(name="xp", bufs=3) as xp,
            tc.tile_pool(name="msgp", bufs=2) as msgp,
            tc.tile_pool(name="sp", bufs=2) as sp,
            tc.tile_pool(name="aggp", bufs=4) as aggp,
            tc.tile_pool(name="hp", bufs=4) as hp,
            tc.tile_pool(name="lnp", bufs=4) as lnp,
            tc.tile_pool(name="ps1", bufs=3, space="PSUM") as ps1,
            tc.tile_pool(name="ps2", bufs=3, space="PSUM") as ps2,
            tc.tile_pool(name="dram", bufs=1, space="DRAM") as dram,
        ):
            # ---- constants / metadata into SBUF ----
            idx_sb = metap.tile([128, total_idxcols], i16)
            nc.sync.dma_start(idx_sb[:], idx_in[:])
            ns_sb = constp.tile([128, WINDOWS], f32)
            nc.sync.dma_start(ns_sb[:], ns_in[:])
            ndn_sb = constp.tile([128, WINDOWS], f32)
            nc.sync.dma_start(ndn_sb[:], ndn_in[:])
            gam_sb = constp.tile([128, D], f32)
            nc.sync.dma_start(gam_sb[:], gam_in[:])
            bet_sb = constp.tile([128, D], f32)
            nc.sync.dma_start(bet_sb[:], bet_in[:])
            w_sb = []
            bb_sb = []
            for i in range(4):
                wt = constp.tile([D, D], f16, name=f"w{i}_sb")
                nc.sync.dma_start(wt[:], w_in[i][:])
                w_sb.append(wt)
                bt = constp.tile([128, D], f32, name=f"bb{i}_sb")
                nc.sync.dma_start(bt[:], bb_in[i][:])
                bb_sb.append(bt)
            eps_t = constp.tile([128, 1], f32)
            nc.vector.memset(eps_t[:], 1e-5)

            # ---- DRAM h buffers ----
            h_shard = [
                dram.tile([NPC, ROWB], fh, name=f"h_shard{l}") for l in range(4)
            ]
            h_bank = [
                [
                    dram.tile([BANK_ROWS, ROWB], fh, addr_space="Shared",
                              name=f"h_bank{l}_{b}")
                    for b in range(BANKS)
                ]
                for l in range(4)
            ]
            rg = [list(range(NCORES))]

            def _ag(l, b):
                nc.gpsimd.collective_compute(
                    "AllGather", mybir.AluOpType.bypass, replica_groups=rg,
                    ins=[h_shard[l][b * CHUNK:(b + 1) * CHUNK, :]],
                    outs=[h_bank[l][b][:]],
                )

            def issue_ag(l, w):
                if w in AG_BOUNDARY:
                    b = AG_BOUNDARY.index(w)
                    if AG_LATE:
                        if b == BANKS - 1:
                            for bb in range(BANKS):
                                _ag(l, bb)
                    else:
                        _ag(l, b)

            # ---- prologue: h_shard0 = x * norm_src (cast fp16) ----
            for w in range(WINDOWS):
                rows = min(128, NPC - w * 128)
                xt = xp.tile([128, D], f32, tag="xt")
                nc.sync.dma_start(xt[:rows], x_in[w * 128:w * 128 + rows, :])
                ht = xp.tile([128, ROWB], fh, tag="ht0")
                nc.scalar.activation(out=ht[:, :D], in_=xt[:], func=Copy,
                                     scale=ns_sb[:, w:w + 1])
                nc.sync.dma_start(h_shard[0][w * 128:w * 128 + rows, :], ht[:rows])
                issue_ag(0, w)

            # ---- layers ----
            for l in range(dbg_layers):
                for si, k in enumerate(sizes):
                    # batched gathers per bank covering k windows, in chunks
                    # of <=8 subtiles (1024 idx: the SWDGE per-op ceiling);
                    # chunks interleave across banks so Q7 never head-of-line
                    # blocks on one queue's ring space
                    msg_t = [
                        msgp.tile([128, K_SS * S * D], fh, tag=f"msg{b}",
                                  name=f"msg{b}")
                        for b in range(BANKS)
                    ]
                    nsub_all = k * S
                    c0 = 0
                    while c0 < nsub_all:
                        nsub_c = min(8, nsub_all - c0)
                        nidx = nsub_c * 128
                        for b in range(BANKS):
                            out_ap = msg_t[b][:, c0 * D:(c0 + nsub_c) * D] \
                                .rearrange("p (k d) -> p k d", d=D)
                            idx_ap = idx_sb[:, off_idx[si][b] + c0 * 8:
                                            off_idx[si][b] + c0 * 8 + nidx // 16]
                            if FP8:
                                _dma_gather_raw(
                                    nc, mybir, out_ap, h_bank[l][b][:], idx_ap,
                                    nidx, D, ROWB, queue_num=b,
                                )
                            else:
                                nc.gpsimd.dma_gather(
                                    out_ap, h_bank[l][b][:], idx_ap,
                                    nidx, nidx, D,
                                    queue_num=b,
                                )
                        c0 += nsub_c
                    # one-hot slab for the whole superstep
                    nsub_ss = k * BANKS * S
                    s_run = sp.tile([128, K_SS * BANKS * S * D], f8, tag="s")
                    nc.sync.dma_start(
                        s_run[:, :nsub_ss * D],
                        s8_in[:, off_sub[si] * D:(off_sub[si] + nsub_ss) * D],
                    )
                    for wl in range(k):
                        w = si * K_SS + wl
                        rows = min(128, NPC - w * 128)
                        psum1 = ps1.tile([128, 128], f32, tag="psum1")
                        n_tot = BANKS * S
                        mi = 0
                        for b in range(BANKS):
                            for s in range(S):
                                nc.tensor.matmul(
                                    psum1[:],
                                    lhsT=msg_t[b][:, (wl * S + s) * D:(wl * S + s + 1) * D],
                                    rhs=s_run[:, ((wl * BANKS + b) * S + s) * D:
                                              ((wl * BANKS + b) * S + s + 1) * D],
                                    start=(mi == 0), stop=(mi == n_tot - 1),
                                )
                                mi += 1
                        # dense: z[dst, of] = aggT.T @ W
                        aggT = aggp.tile([128, 128], f16, tag="aggT")
                        nc.scalar.copy(out=aggT[:], in_=psum1[:])
                        psum2 = ps2.tile([128, 128], f32, tag="psum2")
                        nc.tensor.matmul(psum2[:], lhsT=aggT[:], rhs=w_sb[l][:],
                                         start=True, stop=True)
                        # t2 = norm_dst * z + b  (fused on DVE);
                        # last layer also accumulates sum(t2) for LayerNorm
                        t2 = hp.tile([128, D], f32, tag="t2")
                        last = l == dbg_layers - 1
                        s1 = (lnp.tile([128, 1], f32, tag="s1", name="s1")
                              if last else None)
                        nc.vector.scalar_tensor_tensor(
                            out=t2[:], in0=psum2[:], scalar=ndn_sb[:, w:w + 1],
                            in1=bb_sb[l][:], op0=MUL, op1=ADD,
                            accum_out=s1[:] if last else None,
                        )
                        if l < dbg_layers - 1:
                            g32 = hp.tile([128, D], f32, tag="g32")
                            nc.scalar.activation(out=g32[:], in_=t2[:], func=Gelu)
                            h16 = hp.tile([128, ROWB], fh, tag="h16")
                            nc.vector.tensor_scalar(
                                out=h16[:, :D], in0=g32[:],
                                scalar1=ns_sb[:, w:w + 1], scalar2=None, op0=MUL,
                            )
                            nc.sync.dma_start(
                                h_shard[l + 1][w * 128:w * 128 + rows, :],
                                h16[:rows],
                            )
                            issue_ag(l + 1, w)
                        else:
                            # LayerNorm over features: Σt2 came free from the
                            # STT accum; Σt2² via ACT Square accum
                            sq = lnp.tile([128, D], f32, tag="sq")
                            vs = lnp.tile([128, 1], f32, tag="vs")
                            nc.scalar.activation(out=sq[:], in_=t2[:],
                                                 func=Square,
                                                 accum_out=vs[:])
                            mu = lnp.tile([128, 1], f32, tag="mu")
                            nc.scalar.mul(out=mu[:], in_=s1[:], mul=1.0 / D)
                            mu2 = lnp.tile([128, 1], f32, tag="mu2")
                            nc.vector.tensor_tensor(out=mu2[:], in0=mu[:],
                                                    in1=mu[:], op=MUL)
                            bia = lnp.tile([128, 1], f32, tag="bia")
                            nc.vector.tensor_tensor(out=bia[:], in0=eps_t[:],
                                                    in1=mu2[:], op=SUB)
                            # std = sqrt(E[t2^2] - mu^2 + eps)
                            std = lnp.tile([128, 1], f32, tag="std")
                            nc.scalar.activation(out=std[:], in_=vs[:], func=Sqrt,
                                                 scale=1.0 / D, bias=bia[:])
                            rstd = lnp.tile([128, 1], f32, tag="rstd")
                            nc.vector.reciprocal(out=rstd[:], in_=std[:])
                            cent = lnp.tile([128, D], f32, tag="cent")
                            nc.vector.tensor_scalar(
                                out=cent[:], in0=t2[:], scalar1=mu[:],
                                scalar2=None, op0=SUB,
                            )
                            t4 = lnp.tile([128, D], f32, tag="t4")
                            nc.vector.scalar_tensor_tensor(
                                out=t4[:], in0=cent[:], scalar=rstd[:],
                                in1=gam_sb[:], op0=MUL, op1=MUL,
                            )
                            t5 = lnp.tile([128, D], f32, tag="t5")
                            nc.vector.tensor_tensor(out=t5[:], in0=t4[:],
                                                    in1=bet_sb[:], op=ADD)
                            nc.sync.dma_start(
                                out[w * 128:w * 128 + rows, :], t5[:rows]
                            )
    nc.compile()
    return nc


def kernel(**inputs):
    global LAST_EXEC_NS
    from concourse.bass_utils import run_bass_kernel_spmd

    x = np.asarray(inputs["x"], np.float32)
    src = inputs["src"]
    dst = inputs["dst"]

    key = "prog"
    if key not in _CACHE:
        S, off_idx, off_sub, tic, tsc, per_core, ns_tiles, ndn_tiles = \
            _prep_graph(src, dst)
        nc = _build_program(S, off_idx, off_sub, tic, tsc)
        _CACHE[key] = (nc, per_core, ns_tiles, ndn_tiles)
    nc, per_core, ns_tiles, ndn_tiles = _CACHE[key]

    gamma = np.asarray(inputs["gamma"], np.float32).reshape(1, D)
    beta = np.asarray(inputs["beta"], np.float32).reshape(1, D)
    gamma_b = np.repeat(gamma, 128, axis=0)
    beta_b = np.repeat(beta, 128, axis=0)

    in_maps = []
    for c in range(NCORES):
        idx16, s8 = per_core[c]
        m = {
            "x": np.ascontiguousarray(x[c * NPC:(c + 1) * NPC]),
            "idx16": idx16,
            "s8": s8,
            "ns": ns_tiles[c],
            "ndn": ndn_tiles[c],
            "gamma_b": gamma_b,
            "beta_b": beta_b,
        }
        for i in range(4):
            m[f"W{i+1}"] = np.asarray(inputs[f"W{i+1}"], np.float32).astype(np.float16)
            bb = np.asarray(inputs[f"b{i+1}"], np.float32).reshape(1, D)
            m[f"bb{i+1}"] = np.repeat(bb, 128, axis=0)
        in_maps.append(m)

    if TRACE:
        _install_ntff_hook()
    res = run_bass_kernel_spmd(
        nc, in_maps, core_ids=list(range(NCORES)), trace=TRACE
    )
    LAST_EXEC_NS = res.exec_time_ns
    return np.concatenate(
        [res.results[c]["out"] for c in range(NCORES)], axis=0
    ).astype(np.float32)
